# revision 1
# baseline (speedup 1.0000x reference)
"""Trainium2 Bass kernel for nn_MultiCrossAttention (PVT-style multi-scale
spatial-reduction cross attention).

Sharding: data-parallel over batch (B=32 -> 4 per core x 8 cores), weights
replicated.  All heavy matmuls run in float32r (TF32-like) at 1 cycle/row.

Per-batch pipeline (all "T" tensors are channel-major [c, n]):
  y_i --(contig h-band DMA)--> w-pool tree (DVE/GPSIMD) -> fused
  h-pool+transpose matmuls (PE, pool matrix Ah) -> poolT [c,256] ->
  1x1 conv matmuls (+bias via K=1 matmul) -> channel-major LN (colsum
  matmuls for stats, rank-1 outer-product matmuls for broadcast) -> GELU
  -> xcT.  x -> PE transpose -> xT -> q matmuls -> qT; LN+GELU(xT) -> x4T.
  kv matmuls -> kT (chan-major) + v (token-major, with ones column for
  softmax denominators).  Scores computed TRANSPOSED (sT[nk,nq]) so softmax
  denominator comes free out of the PV matmul's 65th row; normalization via
  reciprocal + rank-1 broadcast.  proj matmuls -> out.
"""

import sys

sys.path.insert(0, "/opt/trn_rl_repo")

import numpy as np

import concourse.bass as bass
import concourse.mybir as mybir
import concourse.tile as tile
from concourse.bass_utils import run_bass_kernel_spmd
from concourse.masks import make_identity

# ---------------------------------------------------------------------------
# Patch: this walrus build only accepts ONE sync-wait on a Drain instruction;
# Tile's tail drain waits on every live semaphore lane.  Split it into a chain
# of single-wait drains.
from concourse.vector_clock import ScopedClock, VectorClock
from concourse.tile_sem_assignment import N_PROCS


def _patched_drain_and_barrier(self, tick_clock, wait_clock):
    nc = self.nc
    gc = tick_clock.global_clock
    nz = [p for p in range(N_PROCS) if gc[p] > 0]
    groups = [nz[i : i + 1] for i in range(0, len(nz), 1)] or [[]]
    for g in groups[:-1]:
        masked = VectorClock([gc[p] if p in g else 0 for p in range(N_PROCS)])
        d = nc.sync.drain()
        wait_clock.add_sem_waits(d.ins, ScopedClock({None: masked}))
    drain_inst = nc.sync.drain()
    last = ScopedClock(
        {None: VectorClock([gc[p] if p in groups[-1] else 0 for p in range(N_PROCS)])}
    )
    wait_clock.add_sem_waits(drain_inst.ins, last)
    nc.all_engine_barrier()
    assert self.sems is not None
    popped = nc._tile_sem_poison_stack.pop()
    assert popped is self._sem_poison
    nc.clear_and_free_semaphores(list(self.sems.allocated().values()))
    nc.all_engine_barrier()


tile.TileContext._drain_and_barrier = _patched_drain_and_barrier


def _split_excess_waits(nc, limit=1):
    """Walrus in this build rejects >2 sync-waits on compute/DMA instructions
    (>1 on Drain).  Move excess waits onto same-engine no-ops inserted just
    before the offending instruction."""
    import bass_rust

    uid = [0]
    for f in nc.m.functions:
        for blk in f.blocks:
            newlist = []
            changed = False
            for ins in blk.instructions:
                si = ins.sync_info
                waits = list(si.on_wait) if si and si.on_wait else []
                tn = type(ins).__name__
                lim = 1 if tn in ("InstDrain", "InstNoOp", "InstTensorTensor") else limit
                if len(waits) > lim:
                    keep = waits[-lim:]
                    for w in waits[:-lim]:
                        nop = bass_rust.InstNoOp(
                            name=f"wsplit-{uid[0]}", ins=[], outs=[]
                        )
                        uid[0] += 1
                        nop.engine = ins.engine
                        nop.sync_info = mybir.SyncInfo(on_wait=[w], on_update=[])
                        newlist.append(nop)
                    ins.sync_info = mybir.SyncInfo(
                        on_wait=keep,
                        on_update=list(si.on_update) if si.on_update else [],
                    )
                    changed = True
                newlist.append(ins)
            if changed:
                blk.instructions = newlist


# ---------------------------------------------------------------------------

F32 = mybir.dt.float32
F32R = mybir.dt.float32r
AF = mybir.ActivationFunctionType

NCORES = 8
B = 32
BPC = B // NCORES  # batches per core
N1 = 256  # query tokens
C1 = 512
NH, HD = 8, 64
SCALE = HD ** -0.5
EPS = 1e-5
C2 = (64, 128, 320)
RATIO = (8, 4, 2)
HW = (128, 64, 32)  # spatial side per branch
GRP = (1, 2, 4)  # w-groups packed into partitions (128 = H*G)
NKV = 256  # kv tokens (16x16 pooled grid for every branch)

# xc channel-permutation: kt bins of 128 rows; each branch ptile lands at a
# 64-aligned partition base.  Global xc order: x1 0:64 | x2 64:192 | x3
# 192:512 | x4 512:1024.
# kt0=[x1 | x3c], kt1=x2, kt2=x3a, kt3=x3b, kt4..7=x4
_PERM = np.concatenate(
    [
        np.arange(0, 64),  # x1        -> kt0[0:64]
        np.arange(448, 512),  # x3 pt2  -> kt0[64:128]
        np.arange(64, 192),  # x2       -> kt1
        np.arange(192, 320),  # x3 pt0  -> kt2
        np.arange(320, 448),  # x3 pt1  -> kt3
        np.arange(512, 1024),  # x4     -> kt4..7
    ]
)
# (kt, base) of each branch ptile in xcT
XC_SLOT = {
    "y1": [(0, 0)],
    "y2": [(1, 0)],
    "y3": [(2, 0), (3, 0), (0, 64)],
    "x4": [(4, 0), (5, 0), (6, 0), (7, 0)],
}


def _pool_mats():
    """Ah matrices: [128, G*16] mapping partition (h,g) -> col (g*16+ho),
    with the full 1/r^2 divisor folded in."""
    out = []
    for i in range(3):
        G, r = GRP[i], RATIO[i]
        H = 128 // G
        m = np.zeros((128, G * 16), dtype=np.float32)
        for h in range(H):
            for g in range(G):
                p = h * G + g
                ho = h // r
                m[p, g * 16 + ho] = 1.0 / (r * r)
        out.append(m)
    return out


def build_module(debug=False, reps=1):
    nc = bass.Bass(trn_type="TRN2")
    dbg_d = {}
    if debug:
        for nm, shp in [
            ("d_poolt1", [64, NKV]), ("d_poolt2", [128, NKV]),
            ("d_poolt3", [128, 3, NKV]), ("d_xT", [128, 4, NKV]),
            ("d_qT", [128, 4, NKV]), ("d_xcT", [128, 8, NKV]),
            ("d_kT", [128, 4, NKV]), ("d_vaug", [128, 2, NH, HD + 1]),
            ("d_ste0", [128, 2, NKV]), ("d_outT", [128, 4, NKV]),
        ]:
            dbg_d[nm] = nc.dram_tensor(nm, shp, F32, kind="ExternalOutput")

    # ---- DRAM I/O -------------------------------------------------------
    x_d = nc.dram_tensor("x", [BPC, N1, C1], F32, kind="ExternalInput")
    y1_d = nc.dram_tensor("y1", [BPC, 128 * 128, 64], F32, kind="ExternalInput")
    y2_d = nc.dram_tensor("y2", [BPC, 64 * 64, 128], F32, kind="ExternalInput")
    y3_d = nc.dram_tensor("y3", [BPC, 32 * 32, 320], F32, kind="ExternalInput")
    wq_d = nc.dram_tensor("wq_t", [C1, C1], F32, kind="ExternalInput")
    wkv_d = nc.dram_tensor("wkv_t", [1024, 1024], F32, kind="ExternalInput")
    proj_d = nc.dram_tensor("proj_t", [C1, C1], F32, kind="ExternalInput")
    projb_d = nc.dram_tensor("projb", [C1], F32, kind="ExternalInput")
    srw_d = [
        nc.dram_tensor(f"srw{i+1}_t", [C2[i], C2[i]], F32, kind="ExternalInput")
        for i in range(3)
    ]
    srb_d = [
        nc.dram_tensor(f"srb{i+1}", [C2[i]], F32, kind="ExternalInput")
        for i in range(3)
    ]
    ah_d = [
        nc.dram_tensor(f"ah{i+1}", [128, GRP[i] * 16], F32, kind="ExternalInput")
        for i in range(3)
    ]
    g_d = [
        nc.dram_tensor(f"g{i+1}", [C2[i]], F32, kind="ExternalInput") for i in range(3)
    ] + [nc.dram_tensor("g4", [C1], F32, kind="ExternalInput")]
    ng_d = [
        nc.dram_tensor(f"ng{i+1}", [C2[i]], F32, kind="ExternalInput")
        for i in range(3)
    ] + [nc.dram_tensor("ng4", [C1], F32, kind="ExternalInput")]
    b_d = [
        nc.dram_tensor(f"lb{i+1}", [C2[i]], F32, kind="ExternalInput")
        for i in range(3)
    ] + [nc.dram_tensor("lb4", [C1], F32, kind="ExternalInput")]
    out_d = nc.dram_tensor("out", [BPC, N1, C1], F32, kind="ExternalOutput")

    CB = [64, 128, 320, 512]  # channels per branch (incl. x4)
    NPT = [1, 1, 3, 4]  # partition tiles per branch

    with tile.TileContext(nc) as tc:
        with (
            tc.tile_pool(name="wts", bufs=1) as wts,
            tc.tile_pool(name="bands", bufs=2) as bandp,
            tc.tile_pool(name="t1", bufs=1) as t1p,
            tc.tile_pool(name="poolt", bufs=1) as pooltp,
            tc.tile_pool(name="bbuf", bufs=1) as bbufp,
            tc.tile_pool(name="work", bufs=1) as work,
            tc.tile_pool(name="rows", bufs=4) as rowsp,
            tc.tile_pool(name="sq", bufs=2) as sqp,
            tc.tile_pool(name="ste", bufs=1) as step,
            tc.tile_pool(name="outb", bufs=1) as outbp,
            tc.tile_pool(name="pp1", bufs=1, space="PSUM") as pp1,
        ):
            # ---- load weights (one-time; gpsimd DMA casts f32 -> f32r) ----
            wq_s = wts.tile([128, 4, C1], F32R)
            nc.gpsimd.dma_start(
                out=wq_s, in_=wq_d.ap().rearrange("(t p) o -> p t o", p=128)
            )
            wkv_s = wts.tile([128, 8, 1024], F32R)
            nc.gpsimd.dma_start(
                out=wkv_s, in_=wkv_d.ap().rearrange("(t p) o -> p t o", p=128)
            )
            proj_s = wts.tile([128, 4, C1], F32R)
            nc.gpsimd.dma_start(
                out=proj_s, in_=proj_d.ap().rearrange("(t p) o -> p t o", p=128)
            )
            projb_s = wts.tile([128, C1], F32)
            nc.scalar.dma_start(
                out=projb_s,
                in_=bass.AP(tensor=projb_d, offset=0, ap=[[0, 128], [1, C1]]),
            )
            srw_s = []
            for i in range(3):
                c = C2[i]
                nkt = (c + 127) // 128
                t = wts.tile([min(c, 128), nkt, c], F32R, tag=f"srw{i}", name=f"srw{i}")
                if c <= 128:
                    nc.gpsimd.dma_start(out=t[:, 0], in_=srw_d[i].ap())
                else:
                    full = (c // 128) * 128
                    nc.gpsimd.dma_start(
                        out=t[:, : c // 128],
                        in_=srw_d[i]
                        .ap()[0:full]
                        .rearrange("(t p) o -> p t o", p=128),
                    )
                    if c % 128:
                        nc.gpsimd.dma_start(
                            out=t[: c % 128, c // 128], in_=srw_d[i].ap()[full:c]
                        )
                srw_s.append(t)
            srb_s = [
                wts.tile([1, C2[i]], F32R, tag=f"srb{i}", name=f"srb{i}")
                for i in range(3)
            ]
            for i in range(3):
                nc.gpsimd.dma_start(
                    out=srb_s[i],
                    in_=bass.AP(tensor=srb_d[i], offset=0, ap=[[0, 1], [1, C2[i]]]),
                )
            ah_s = []
            for i in range(3):
                t = wts.tile([128, GRP[i] * 16], F32R, tag=f"ah{i}", name=f"ah{i}")
                nc.gpsimd.dma_start(out=t, in_=ah_d[i].ap())
                ah_s.append(t)
            g_s, ng_s, b_s = [], [], []
            for i in range(4):
                c = CB[i]
                gt = wts.tile([1, c], F32R, tag=f"g{i}", name=f"g{i}")
                ngt = wts.tile([1, c], F32R, tag=f"ng{i}", name=f"ng{i}")
                bt = wts.tile([1, c], F32R, tag=f"b{i}", name=f"b{i}")
                nc.gpsimd.dma_start(
                    out=gt, in_=bass.AP(tensor=g_d[i], offset=0, ap=[[0, 1], [1, c]])
                )
                nc.gpsimd.dma_start(
                    out=ngt, in_=bass.AP(tensor=ng_d[i], offset=0, ap=[[0, 1], [1, c]])
                )
                nc.gpsimd.dma_start(
                    out=bt, in_=bass.AP(tensor=b_d[i], offset=0, ap=[[0, 1], [1, c]])
                )
                g_s.append(gt)
                ng_s.append(ngt)
                b_s.append(bt)

            ident = wts.tile([128, 128], F32)
            make_identity(nc, ident)
            onescol = wts.tile([128, 1], F32R)
            nc.vector.memset(onescol.bitcast(F32), 1.0)
            onesrow = wts.tile([1, NKV], F32R)
            nc.vector.memset(onesrow.bitcast(F32), 1.0)
            ones64 = wts.tile([1, 64], F32R)
            nc.vector.memset(ones64.bitcast(F32), 1.0)
            epsrow = wts.tile([1, 1], F32)
            nc.gpsimd.memset(epsrow, EPS)

            y1r = y1_d.ap().rearrange("b (h w) c -> b h (w c)", h=128)
            y2r = y2_d.ap().rearrange("b (h wb wi) c -> b (h wb) (wi c)", wb=2, wi=32)
            y3r = y3_d.ap().rearrange("b (h wb wi) c -> b (h wb) (wi c)", wb=4, wi=8)
            xr = x_d.ap().rearrange("b (nt p) c -> b p nt c", p=128)
            outr = out_d.ap().rearrange("b (nt p) c -> b p nt c", p=128)

            for rep in range(reps):
             for bi in range(BPC):
                # ==== x: load + transpose -> xT; q matmuls ==============
                x_sb = work.tile([128, 2, C1], F32, tag="x_sb")
                nc.sync.dma_start(out=x_sb, in_=xr[bi])
                xT = work.tile([128, 4, NKV], F32R, tag="xT", bufs=2)
                for cc in range(4):
                    tp = pp1.tile([128, 2, 128], F32, tag="ppB", name="xtp", bufs=2)
                    for nt in range(2):
                        nc.tensor.transpose(
                            tp[:, nt], x_sb[:, nt, cc * 128 : (cc + 1) * 128], ident
                        )
                    nc.scalar.copy(out=xT[:, cc], in_=tp.rearrange("p a b -> p (a b)"))

                qT = work.tile([128, 4, NKV], F32R, tag="qT", bufs=2)
                for mt in range(4):
                    qp = pp1.tile([128, NKV], F32, tag="ppB", name="qp", bufs=2)
                    for kt in range(4):
                        nc.tensor.matmul(
                            qp,
                            wq_s[:, kt, mt * 128 : (mt + 1) * 128],
                            xT[:, kt],
                            start=(kt == 0),
                            stop=(kt == 3),
                        )
                    nc.scalar.copy(out=qT[:, mt], in_=qp)

                # ==== branch pooling ====================================
                # ---- y1: two halves, 8->1 w-tree (3 levels) ----
                t1y1 = t1p.tile([128, 16, 64], F32R, tag="t1y1")
                for qt in range(4):
                    band = bandp.tile([128, 2048], F32, tag="bandq", bufs=2)
                    nc.sync.dma_start(
                        out=band, in_=y1r[bi, :, qt * 2048 : (qt + 1) * 2048]
                    )
                    v = band.rearrange("p (wo dw c) -> p wo dw c", wo=4, dw=8)
                    nc.gpsimd.tensor_add(v[:, :, 0:4], v[:, :, 0:4], v[:, :, 4:8])
                    nc.vector.tensor_add(v[:, :, 0:2], v[:, :, 0:2], v[:, :, 2:4])
                    nc.vector.tensor_add(
                        t1y1[:, qt * 4 : (qt + 1) * 4], v[:, :, 0], v[:, :, 1]
                    )
                # ---- y2: two half-bands, 4->1 tree (2 levels) ----
                t1y2 = t1p.tile([128, 8, 128], F32R, tag="t1y2")
                for hf in range(2):
                    band = bandp.tile([128, 2048], F32, tag="band2", name="band2", bufs=2)
                    nc.sync.dma_start(
                        out=band, in_=y2r[bi, :, hf * 2048 : (hf + 1) * 2048]
                    )
                    v = band.rearrange("p (wo dw c) -> p wo dw c", wo=4, dw=4)
                    nc.gpsimd.tensor_add(v[:, :, 0:2], v[:, :, 0:2], v[:, :, 2:4])
                    nc.vector.tensor_add(
                        t1y2[:, hf * 4 : (hf + 1) * 4], v[:, :, 0], v[:, :, 1]
                    )
                # ---- y3: single band, 2->1 tree ----
                t1y3 = t1p.tile([128, 4, 320], F32R, tag="t1y3")
                band3 = bandp.tile([128, 2560], F32, tag="band3", bufs=1)
                v3f = band3.rearrange("p (wo dw c) -> p wo dw c", wo=4, dw=2)
                nc.sync.dma_start(out=band3, in_=y3r[bi])
                nc.gpsimd.tensor_add(t1y3, v3f[:, :, 0], v3f[:, :, 1])

                # ---- fused h-pool + transpose (PE) -> poolT ----
                poolp1 = pp1.tile([64, 16, 16], F32, tag="ppA", name="poolp1", bufs=3)
                for wo in range(16):
                    nc.tensor.matmul(
                        poolp1[:, wo], t1y1[:, wo], ah_s[0], start=True, stop=True
                    )
                poolt1 = pooltp.tile([64, NKV], F32R, tag="poolt1")
                nc.scalar.copy(out=poolt1, in_=poolp1.rearrange("c a b -> c (a b)"))

                poolp2 = pp1.tile([128, 2, 8, 16], F32, tag="ppA", name="poolp2", bufs=3)
                for wo in range(8):
                    nc.tensor.matmul(
                        poolp2[:, :, wo], t1y2[:, wo], ah_s[1], start=True, stop=True
                    )
                poolt2 = pooltp.tile([128, NKV], F32R, tag="poolt2")
                nc.scalar.copy(out=poolt2, in_=poolp2.rearrange("c g a b -> c (g a b)"))

                poolt3 = pooltp.tile([128, 3, NKV], F32R, tag="poolt3")
                for cs in range(3):
                    cl = 64 if cs == 2 else 128
                    poolp3 = pp1.tile([128, 4, 4, 16], F32, tag="ppA", name="poolp3", bufs=3)
                    for wo in range(4):
                        nc.tensor.matmul(
                            poolp3[:cl, :, wo],
                            t1y3[:, wo, cs * 128 : cs * 128 + cl],
                            ah_s[2],
                            start=True,
                            stop=True,
                        )
                    nc.scalar.copy(
                        out=poolt3[:cl, cs],
                        in_=poolp3[:cl].rearrange("c g a b -> c (g a b)"),
                    )

                # ==== branch conv + LN + GELU -> xcT ====================
                xcT = work.tile([128, 8, NKV], F32R, tag="xcT")
                poolts = [poolt1, poolt2, poolt3]

                for br in [3, 1, 2, 0]:
                    cb = CB[br]
                    npt = NPT[br]
                    # conv -> preP psum tiles (list per ptile), or x4: use xT
                    datas = []  # sbuf fp32r [cpt, 256] data tiles per ptile
                    if br < 3:
                        bb = bbufp.tile([128, npt, NKV], F32R, tag=f"bb{br}", name=f"bb{br}")
                        sqs = []
                        for pt in range(npt):
                            cl = min(128, cb - pt * 128)
                            prep = pp1.tile([128, NKV], F32, tag="ppA", name="prep", bufs=3)
                            nkt = (cb + 127) // 128
                            for kt in range(nkt):
                                kl = min(128, cb - kt * 128)
                                if br < 2:
                                    lhs = srw_s[br][
                                        :kl, 0, pt * 128 : pt * 128 + cl
                                    ]
                                    rhs = poolts[br][:kl]
                                else:
                                    lhs = srw_s[2][:kl, kt, pt * 128 : pt * 128 + cl]
                                    rhs = poolts[2][:kl, kt]
                                nc.tensor.matmul(
                                    prep[:cl], lhs, rhs, start=(kt == 0), stop=False
                                )
                            # bias via K=1 matmul with ones row
                            nc.tensor.matmul(
                                prep[:cl],
                                srb_s[br][:, pt * 128 : pt * 128 + cl],
                                onesrow,
                                start=False,
                                stop=True,
                            )
                            nc.scalar.copy(out=bb[:cl, pt], in_=prep[:cl])
                            sq = sqp.tile([128, NKV], F32R, tag="x4sq", name="bsq")
                            nc.scalar.activation(
                                out=sq[:cl], in_=prep[:cl], func=AF.Square
                            )
                            sqs.append(sq)
                            datas.append(bb[:cl, pt])
                    else:
                        x4sqs = []
                        for kt in range(4):
                            sq = sqp.tile([128, NKV], F32R, tag="x4sq")
                            nc.scalar.activation(
                                out=sq, in_=xT[:, kt], func=AF.Square
                            )
                            x4sqs.append(sq)
                            datas.append(xT[:, kt])
                        stat_rhs = None

                    # column sums (PE): accumulate over ptiles
                    stats = pp1.tile([1, 2, NKV], F32, tag="ppA", name="stats", bufs=3)
                    if br < 3:
                        for pt in range(npt):
                            cl = min(128, cb - pt * 128)
                            nc.tensor.matmul(
                                stats[:, 0],
                                onescol[:cl],
                                bb[:cl, pt],
                                start=(pt == 0),
                                stop=(pt == npt - 1),
                                skip_group_check=True,
                            )
                        for pt in range(npt):
                            cl = min(128, cb - pt * 128)
                            nc.tensor.matmul(
                                stats[:, 1],
                                onescol[:cl],
                                sqs[pt][:cl],
                                start=(pt == 0),
                                stop=(pt == npt - 1),
                                skip_group_check=True,
                            )
                    else:
                        for kt in range(4):
                            nc.tensor.matmul(
                                stats[:, 0],
                                onescol,
                                datas[kt],
                                start=(kt == 0),
                                stop=(kt == 3),
                                skip_group_check=True,
                            )
                        for kt in range(4):
                            nc.tensor.matmul(
                                stats[:, 1],
                                onescol,
                                x4sqs[kt],
                                start=(kt == 0),
                                stop=(kt == 3),
                                skip_group_check=True,
                            )

                    # stats -> mean / rstd rows
                    mrow = rowsp.tile([1, NKV], F32, tag="tmprow", name="mrow")
                    r1row = rowsp.tile([1, NKV], F32, tag="tmprow", name="r1row")
                    nc.vector.tensor_scalar_mul(mrow, stats[:, 0], 1.0 / cb)
                    nc.vector.tensor_scalar_mul(r1row, stats[:, 1], 1.0 / cb)
                    msq = rowsp.tile([1, NKV], F32, tag="tmprow", name="msq")
                    nc.scalar.activation(out=msq, in_=mrow, func=AF.Square)
                    var = rowsp.tile([1, NKV], F32, tag="tmprow", name="var")
                    nc.vector.tensor_sub(var, r1row, msq)
                    sd = rowsp.tile([1, NKV], F32, tag="tmprow", name="sd")
                    nc.scalar.activation(
                        out=sd, in_=var, func=AF.Sqrt, bias=epsrow
                    )
                    rstd_r = rowsp.tile([1, NKV], F32R, tag="rstd_r", bufs=2)
                    with nc.allow_low_precision(reason="tf32 rstd is fine"):
                        nc.vector.reciprocal(rstd_r, sd)
                    mr_r = rowsp.tile([1, NKV], F32R, tag="mr_r", bufs=2)
                    nc.vector.tensor_mul(mr_r, mrow, rstd_r)

                    # rank-1 broadcasts + normalize + gelu into xcT slots
                    for pt in range(NPT[br]):
                        cl = min(128, cb - pt * 128)
                        kt_slot, base = XC_SLOT[["y1", "y2", "y3", "x4"][br]][pt]
                        S = pp1.tile([128, NKV], F32, tag="ppA", name="Sbc", bufs=3)
                        nc.tensor.matmul(
                            S[:cl],
                            g_s[br][:, pt * 128 : pt * 128 + cl],
                            rstd_r,
                            start=True,
                            stop=True,
                        )
                        Bb = pp1.tile([128, NKV], F32, tag="ppA", name="Bbc", bufs=3)
                        nc.tensor.matmul(
                            Bb[:cl],
                            b_s[br][:, pt * 128 : pt * 128 + cl],
                            onesrow,
                            start=True,
                            stop=False,
                        )
                        nc.tensor.matmul(
                            Bb[:cl],
                            ng_s[br][:, pt * 128 : pt * 128 + cl],
                            mr_r,
                            start=False,
                            stop=True,
                        )
                        dst = xcT[base : base + cl, kt_slot]
                        if br < 3:
                            # branch data is already in tau=(wo*16+ho) order
                            nc.vector.tensor_mul(dst, datas[pt], S[:cl])
                            nc.vector.tensor_add(dst, dst, Bb[:cl])
                            nc.scalar.activation(out=dst, in_=dst, func=AF.Gelu)
                        else:
                            # x4 tokens are in natural (ho*16+wo) order; permute
                            # the gelu's write AP to tau so all kv channels of a
                            # token refer to the same spatial position.
                            tmpn = sqp.tile(
                                [128, NKV], F32R, tag="nrm", name="nrm"
                            )
                            nc.vector.tensor_mul(tmpn, datas[pt], S[:cl])
                            nc.vector.tensor_add(tmpn, tmpn, Bb[:cl])
                            nc.scalar.activation(
                                out=dst.rearrange(
                                    "c (wo ho) -> c ho wo", wo=16
                                ),
                                in_=tmpn.rearrange(
                                    "c (ho wo) -> c ho wo", ho=16
                                ),
                                func=AF.Gelu,
                            )

                # ==== kv matmuls ========================================
                KTORD = [4, 5, 6, 7, 1, 2, 3, 0]
                kT = work.tile([128, 4, NKV], F32R, tag="kT")
                for mt in range(4):
                    kp = pp1.tile([128, NKV], F32, tag="ppB", name="kp", bufs=2)
                    for i, kt in enumerate(KTORD):
                        nc.tensor.matmul(
                            kp,
                            wkv_s[:, kt, mt * 128 : (mt + 1) * 128],
                            xcT[:, kt],
                            start=(i == 0),
                            stop=(i == 7),
                        )
                    nc.scalar.copy(out=kT[:, mt], in_=kp)

                v_aug = work.tile([128, 2, NH, HD + 1], F32R, tag="v_aug", bufs=2)
                nc.vector.memset(v_aug[:, :, :, HD : HD + 1].bitcast(F32), 1.0)
                for mt in range(2):
                    vp = pp1.tile([128, C1], F32, tag="ppB", name="vp", bufs=2)
                    for i, kt in enumerate(KTORD):
                        nc.tensor.matmul(
                            vp,
                            xcT[:, kt, mt * 128 : (mt + 1) * 128],
                            wkv_s[:, kt, 512:1024],
                            start=(i == 0),
                            stop=(i == 7),
                        )
                    nc.scalar.copy(
                        out=v_aug[:, mt, :, 0:HD],
                        in_=vp.rearrange("p (h d) -> p h d", h=NH),
                    )

                # ==== attention per head ================================
                outT = work.tile([128, 4, NKV], F32R, tag="outT", bufs=2)
                for h in range(NH):
                    pb = (h % 2) * 64
                    ck = h // 2
                    sp = pp1.tile([128, 2, NKV], F32, tag="ppC", name="sp", bufs=2)
                    for nt in range(2):
                        nc.tensor.matmul(
                            sp[:, nt],
                            kT[pb : pb + 64, ck, nt * 128 : (nt + 1) * 128],
                            qT[pb : pb + 64, ck],
                            start=True,
                            stop=True,
                        )
                    ste = step.tile([128, 2, NKV], F32R, tag="ste")
                    nc.scalar.activation(out=ste, in_=sp, func=AF.Exp, scale=SCALE)
                    if debug and bi == 0 and h == 0:
                        nc.sync.dma_start(out=dbg_d["d_ste0"].ap(), in_=ste.bitcast(F32))
                    pv = pp1.tile([65, NKV], F32, tag="ppC", name="pv", bufs=2)
                    for nt in range(2):
                        nc.tensor.matmul(
                            pv,
                            v_aug[:, nt, h],
                            ste[:, nt],
                            start=(nt == 0),
                            stop=(nt == 1),
                        )
                    rs_r = rowsp.tile([1, NKV], F32R, tag="rs_r", bufs=2)
                    with nc.allow_low_precision(reason="tf32 softmax denom"):
                        nc.vector.reciprocal(rs_r, pv[64:65])
                    bc = pp1.tile([64, NKV], F32, tag="ppC", name="bc", bufs=2)
                    nc.tensor.matmul(bc, ones64, rs_r, start=True, stop=True)
                    bcs = step.tile([64, NKV], F32, tag="bcs")
                    nc.scalar.copy(out=bcs, in_=bc)
                    nc.vector.tensor_mul(outT[pb : pb + 64, ck], pv[0:64], bcs)

                # ==== proj + bias + store ===============================
                osb = outbp.tile([128, 2, C1], F32, tag="osb")
                for nt in range(2):
                    fp = pp1.tile([128, C1], F32, tag="ppD", name="fp", bufs=1)
                    for kt in range(4):
                        nc.tensor.matmul(
                            fp,
                            outT[:, kt, nt * 128 : (nt + 1) * 128],
                            proj_s[:, kt],
                            start=(kt == 0),
                            stop=(kt == 3),
                        )
                    nc.vector.tensor_add(osb[:, nt], fp, projb_s)
                nc.sync.dma_start(out=outr[bi], in_=osb)
                if debug and bi == 0:
                    for nm, tl in [
                        ("d_poolt1", poolt1), ("d_poolt2", poolt2),
                        ("d_poolt3", poolt3), ("d_xT", xT), ("d_qT", qT),
                        ("d_xcT", xcT), ("d_kT", kT), ("d_vaug", v_aug),
                        ("d_outT", outT),
                    ]:
                        nc.sync.dma_start(
                            out=dbg_d[nm].ap(), in_=tl.bitcast(F32)
                        )

    _split_excess_waits(nc)
    return nc


def kernel(**inputs):
    x = np.ascontiguousarray(inputs["x"], dtype=np.float32)
    y1 = np.ascontiguousarray(inputs["y1"], dtype=np.float32)
    y2 = np.ascontiguousarray(inputs["y2"], dtype=np.float32)
    y3 = np.ascontiguousarray(inputs["y3"], dtype=np.float32)
    Wq = np.asarray(inputs["Wq"], dtype=np.float32)
    Wkv = np.asarray(inputs["Wkv"], dtype=np.float32)
    proj_w = np.asarray(inputs["proj_w"], dtype=np.float32)
    proj_b = np.asarray(inputs["proj_b"], dtype=np.float32)

    wq_t = np.ascontiguousarray(Wq.T)
    wkv_t = np.ascontiguousarray(Wkv.T[_PERM, :])
    proj_t = np.ascontiguousarray(proj_w.T)
    ah = _pool_mats()

    common = {
        "wq_t": wq_t,
        "wkv_t": wkv_t,
        "proj_t": proj_t,
        "projb": proj_b,
        "ah1": ah[0],
        "ah2": ah[1],
        "ah3": ah[2],
    }
    for i in range(3):
        common[f"srw{i+1}_t"] = np.ascontiguousarray(
            np.asarray(inputs[f"sr{i+1}_w"], dtype=np.float32).T
        )
        common[f"srb{i+1}"] = np.asarray(inputs[f"sr{i+1}_b"], dtype=np.float32)
        g = np.asarray(inputs[f"ln{i+1}_g"], dtype=np.float32)
        common[f"g{i+1}"] = g
        common[f"ng{i+1}"] = -g
        common[f"lb{i+1}"] = np.asarray(inputs[f"ln{i+1}_b"], dtype=np.float32)
    g4 = np.asarray(inputs["ln4_g"], dtype=np.float32)
    common["g4"] = g4
    common["ng4"] = -g4
    common["lb4"] = np.asarray(inputs["ln4_b"], dtype=np.float32)

    nc = build_module()
    in_maps = []
    for c in range(NCORES):
        sl = slice(c * BPC, (c + 1) * BPC)
        m = dict(common)
        m["x"] = x[sl]
        m["y1"] = y1[sl]
        m["y2"] = y2[sl]
        m["y3"] = y3[sl]
        in_maps.append(m)

    res = run_bass_kernel_spmd(nc, in_maps, core_ids=list(range(NCORES)))
    return np.concatenate([r["out"] for r in res.results], axis=0)


if __name__ == "__main__":
    pass



# revision 3
# speedup vs baseline: 45452.6253x; 45452.6253x over previous
"""Trainium2 Bass kernel for nn_MultiCrossAttention (PVT-style multi-scale
spatial-reduction cross attention) — v2.

Sharding: data-parallel over batch (B=32 -> 4 per core x 8 cores), weights
replicated.  All inputs are cast to bf16 on the host (tolerance is 2e-2; bf16
keeps us ~5e-3) which halves HBM traffic — the memory roofline.

Per-batch pipeline:
  y_i --(contig band DMA, bf16)--> w-pool tree (DVE adds) -> fused
  h-pool+transpose matmuls (PE, pool matrix Ah) -> poolT [c,256] (chan-major).
  Conv runs TOKEN-major: out[tok, c_out] = poolT-chunk^T @ srwT-chunk (+bias
  via K=1 ones-row matmul).  LN stats are then free-axis reductions
  (tensor_reduce / stt accum_out) giving per-token mean/var COLUMNS;
  rstd = exp(-0.5*ln(var+eps)) on the Act engine (Ln+Exp share one
  activation table with the attention Exp — 2 table loads per batch).
  Normalize = (conv - m)*rstd via per-partition tensor_scalar, transpose
  back to chan-major on the PE, and GELU reads the transpose PSUM directly
  with gamma/beta folded into the Act op's per-partition scale/bias.
  x: PE transpose -> xT -> q matmuls; x4 branch same token-major LN trick.
  kv matmuls -> kT (chan-major) + v_aug (token-major, ones column for the
  softmax denominator).  Scores TRANSPOSED (sT[kv,q]) so the denominator
  falls out of the PV matmul's 65th row; normalization via reciprocal +
  rank-1 ones2 broadcast (two heads per matmul) + fused scalar_tensor_tensor.
  proj matmuls (token-major) -> + bias -> out.
"""

import sys

sys.path.insert(0, "/opt/trn_rl_repo")

import numpy as np
import ml_dtypes

import concourse.bass as bass
import concourse.mybir as mybir
import concourse.tile as tile
from concourse.bass_utils import run_bass_kernel_spmd
from concourse.masks import make_identity

# ---------------------------------------------------------------------------
# Patch: this walrus build only accepts ONE sync-wait on a Drain instruction;
# Tile's tail drain waits on every live semaphore lane.  Split it into a chain
# of single-wait drains.
from concourse.vector_clock import ScopedClock, VectorClock
from concourse.tile_sem_assignment import N_PROCS


def _patched_drain_and_barrier(self, tick_clock, wait_clock):
    nc = self.nc
    gc = tick_clock.global_clock
    nz = [p for p in range(N_PROCS) if gc[p] > 0]
    groups = [nz[i : i + 1] for i in range(0, len(nz), 1)] or [[]]
    for g in groups[:-1]:
        masked = VectorClock([gc[p] if p in g else 0 for p in range(N_PROCS)])
        d = nc.sync.drain()
        wait_clock.add_sem_waits(d.ins, ScopedClock({None: masked}))
    drain_inst = nc.sync.drain()
    last = ScopedClock(
        {None: VectorClock([gc[p] if p in groups[-1] else 0 for p in range(N_PROCS)])}
    )
    wait_clock.add_sem_waits(drain_inst.ins, last)
    nc.all_engine_barrier()
    assert self.sems is not None
    popped = nc._tile_sem_poison_stack.pop()
    assert popped is self._sem_poison
    nc.clear_and_free_semaphores(list(self.sems.allocated().values()))
    nc.all_engine_barrier()


tile.TileContext._drain_and_barrier = _patched_drain_and_barrier


def _split_excess_waits(nc, limit=1):
    """Walrus in this build rejects >2 sync-waits on compute/DMA instructions
    (>1 on Drain).  Move excess waits onto same-engine no-ops inserted just
    before the offending instruction."""
    import bass_rust

    uid = [0]
    for f in nc.m.functions:
        for blk in f.blocks:
            newlist = []
            changed = False
            for ins in blk.instructions:
                si = ins.sync_info
                waits = list(si.on_wait) if si and si.on_wait else []
                tn = type(ins).__name__
                lim = 1 if tn in ("InstDrain", "InstNoOp", "InstTensorTensor") else limit
                if len(waits) > lim:
                    keep = waits[-lim:]
                    for w in waits[:-lim]:
                        nop = bass_rust.InstNoOp(
                            name=f"wsplit-{uid[0]}", ins=[], outs=[]
                        )
                        uid[0] += 1
                        nop.engine = ins.engine
                        nop.sync_info = mybir.SyncInfo(on_wait=[w], on_update=[])
                        newlist.append(nop)
                    ins.sync_info = mybir.SyncInfo(
                        on_wait=keep,
                        on_update=list(si.on_update) if si.on_update else [],
                    )
                    changed = True
                newlist.append(ins)
            if changed:
                blk.instructions = newlist


# ---------------------------------------------------------------------------

F32 = mybir.dt.float32
BF16 = mybir.dt.bfloat16
AF = mybir.ActivationFunctionType
ALU = mybir.AluOpType

NCORES = 8
B = 32
BPC = B // NCORES  # batches per core
N1 = 256  # query tokens
C1 = 512
NH, HD = 8, 64
SCALE = HD ** -0.5
EPS = 1e-5
C2 = (64, 128, 320)
RATIO = (8, 4, 2)
GRP = (1, 2, 4)  # w-groups packed into partitions (128 = H*G)
NKV = 256  # kv tokens (16x16 pooled grid for every branch)

# xc channel-permutation: kt bins of 128 rows; each branch ptile lands at a
# 64-aligned partition base.  Global xc order: x1 0:64 | x2 64:192 | x3
# 192:512 | x4 512:1024.
# kt0=[x1 | x3c], kt1=x2, kt2=x3a, kt3=x3b, kt4..7=x4
_PERM = np.concatenate(
    [
        np.arange(0, 64),  # x1        -> kt0[0:64]
        np.arange(448, 512),  # x3 pt2  -> kt0[64:128]
        np.arange(64, 192),  # x2       -> kt1
        np.arange(192, 320),  # x3 pt0  -> kt2
        np.arange(320, 448),  # x3 pt1  -> kt3
        np.arange(512, 1024),  # x4     -> kt4..7
    ]
)


def _pool_mats():
    """Ah matrices: [128, G*16] mapping partition (h,g) -> col (g*16+ho),
    with the full 1/r^2 divisor folded in."""
    out = []
    for i in range(3):
        G, r = GRP[i], RATIO[i]
        H = 128 // G
        m = np.zeros((128, G * 16), dtype=np.float32)
        for h in range(H):
            for g in range(G):
                p = h * G + g
                ho = h // r
                m[p, g * 16 + ho] = 1.0 / (r * r)
        out.append(m)
    return out


ABLATE = set()


def build_module(reps=1):
    nc = bass.Bass(trn_type="TRN2")

    # ---- DRAM I/O -------------------------------------------------------
    x_d = nc.dram_tensor("x", [BPC, N1, C1], BF16, kind="ExternalInput")
    y1_d = nc.dram_tensor("y1", [BPC, 128 * 128, 64], BF16, kind="ExternalInput")
    y2_d = nc.dram_tensor("y2", [BPC, 64 * 64, 128], BF16, kind="ExternalInput")
    y3_d = nc.dram_tensor("y3", [BPC, 32 * 32, 320], BF16, kind="ExternalInput")
    wq_d = nc.dram_tensor("wq_t", [C1, C1], BF16, kind="ExternalInput")
    wkv_d = nc.dram_tensor("wkv_t", [1024, 1024], BF16, kind="ExternalInput")
    proj_d = nc.dram_tensor("proj_t", [C1, C1], BF16, kind="ExternalInput")
    projb_d = nc.dram_tensor("projb", [C1], BF16, kind="ExternalInput")
    srw_d = [
        nc.dram_tensor(
            f"srw{i+1}_t",
            [((C2[i] + 127) // 128) * min(C2[i], 128), C2[i]],
            BF16,
            kind="ExternalInput",
        )
        for i in range(3)
    ]
    srb_d = [
        nc.dram_tensor(f"srb{i+1}", [C2[i]], BF16, kind="ExternalInput")
        for i in range(3)
    ]
    ah_d = [
        nc.dram_tensor(f"ah{i+1}", [128, GRP[i] * 16], BF16, kind="ExternalInput")
        for i in range(3)
    ]
    # gamma / beta packed host-side as [nch*128] padded columns
    CB = [64, 128, 320, 512]  # channels per branch (incl. x4)
    NCH = [1, 1, 3, 4]  # 128-channel chunks per branch
    g_d = [
        nc.dram_tensor(f"g{i+1}", [NCH[i] * 128], F32, kind="ExternalInput")
        for i in range(4)
    ]
    b_d = [
        nc.dram_tensor(f"lb{i+1}", [NCH[i] * 128], F32, kind="ExternalInput")
        for i in range(4)
    ]
    out_d = nc.dram_tensor("out", [BPC, N1, C1], BF16, kind="ExternalOutput")

    NPT = [1, 1, 3, 4]  # partition tiles per branch in xcT
    # (kt, base) of each branch ptile in xcT
    XC_SLOT = {
        0: [(0, 0)],
        1: [(1, 0)],
        2: [(2, 0), (3, 0), (0, 64)],
        3: [(4, 0), (5, 0), (6, 0), (7, 0)],
    }

    with tile.TileContext(nc) as tc:
        with (
            tc.tile_pool(name="wts", bufs=1) as wts,
            tc.tile_pool(name="bands", bufs=9) as bandp,
            tc.tile_pool(name="t1", bufs=2) as t1p,
            tc.tile_pool(name="poolt", bufs=2) as pooltp,
            tc.tile_pool(name="work", bufs=2) as work,
            tc.tile_pool(name="xn", bufs=2) as xnp,
            tc.tile_pool(name="scrap", bufs=4) as scrapp,
            tc.tile_pool(name="cols", bufs=2) as colsp,
            tc.tile_pool(name="rows", bufs=2) as rowsp,
            tc.tile_pool(name="ste", bufs=2) as step,
            tc.tile_pool(name="pp", bufs=1, space="PSUM") as pp,
        ):
            # ---- load weights: small/pool-critical first, big GEMM weights last
            srw_s = []
            for i in range(3):
                c = C2[i]
                nkt = (c + 127) // 128
                t = wts.tile([min(c, 128), nkt, c], BF16, tag=f"srw{i}", name=f"srw{i}")
                nc.scalar.dma_start(
                    out=t, in_=srw_d[i].ap().rearrange("(t p) o -> p t o", p=min(c, 128))
                )
                srw_s.append(t)
            srb_s = [
                wts.tile([1, C2[i]], BF16, tag=f"srb{i}", name=f"srb{i}")
                for i in range(3)
            ]
            for i in range(3):
                nc.scalar.dma_start(
                    out=srb_s[i],
                    in_=bass.AP(tensor=srb_d[i], offset=0, ap=[[0, 1], [1, C2[i]]]),
                )
            ah_s = []
            for i in range(3):
                t = wts.tile([128, GRP[i] * 16], BF16, tag=f"ah{i}", name=f"ah{i}")
                nc.scalar.dma_start(out=t, in_=ah_d[i].ap())
                ah_s.append(t)
            g_s, b_s = [], []
            for i in range(4):
                gt = wts.tile([128, NCH[i]], F32, tag=f"g{i}", name=f"g{i}")
                bt = wts.tile([128, NCH[i]], F32, tag=f"b{i}", name=f"b{i}")
                nc.scalar.dma_start(
                    out=gt,
                    in_=bass.AP(tensor=g_d[i], offset=0, ap=[[1, 128], [128, NCH[i]]]),
                )
                nc.scalar.dma_start(
                    out=bt,
                    in_=bass.AP(tensor=b_d[i], offset=0, ap=[[1, 128], [128, NCH[i]]]),
                )
                g_s.append(gt)
                b_s.append(bt)

            wq_s = wts.tile([128, 4, C1], BF16)
            nc.scalar.dma_start(
                out=wq_s, in_=wq_d.ap().rearrange("(t p) o -> p t o", p=128)
            )
            wkv_s = wts.tile([128, 8, 1024], BF16)
            nc.scalar.dma_start(
                out=wkv_s, in_=wkv_d.ap().rearrange("(t p) o -> p t o", p=128)
            )
            proj_s = wts.tile([128, 4, C1], BF16)
            nc.scalar.dma_start(
                out=proj_s, in_=proj_d.ap().rearrange("(t p) o -> p t o", p=128)
            )
            projb_s = wts.tile([128, C1], BF16)
            nc.scalar.dma_start(
                out=projb_s,
                in_=bass.AP(tensor=projb_d, offset=0, ap=[[0, 128], [1, C1]]),
            )
            ident = wts.tile([128, 128], BF16)
            make_identity(nc, ident)
            onesrow = wts.tile([1, 128], BF16)
            nc.vector.memset(onesrow, 1.0)
            epscol = wts.tile([128, 1], F32)
            nc.gpsimd.memset(epscol, EPS)

            y1r = y1_d.ap().rearrange("b (h w) c -> b h (w c)", h=128)
            y2r = y2_d.ap().rearrange("b (h wb wi) c -> b (h wb) (wi c)", wb=2, wi=32)
            y3r = y3_d.ap().rearrange("b (h wb wi) c -> b (h wb) (wi c)", wb=4, wi=8)
            xr = x_d.ap().rearrange("b (nt p) c -> b p nt c", p=128)
            outr = out_d.ap().rearrange("b (nt p) c -> b p nt c", p=128)

            def s1_gen(bi, st):
                """Loads + PE pooling + conv + bn-stats LN + normalize."""
                x_sb = work.tile([128, 2, C1], BF16, tag="x_sb")
                nc.sync.dma_start(out=x_sb, in_=xr[bi])
                # x4 stats via bn_stats (free-axis mean/var per token)
                bst4 = colsp.tile([128, 2, 6], F32, tag="bst4", name="bst4")
                mv4 = colsp.tile([128, 2, 2], F32, tag="mv4", name="mv4")
                for nt in range(2):
                    nc.vector.bn_stats(bst4[:, nt], x_sb[:, nt])
                    nc.vector.bn_aggr(mv4[:, nt], bst4[:, nt])
                yield

                # ---- band loads; y1 gets a DVE first-level (dw 8->4) ----
                vb1 = []
                for qt in range(4):
                    band = bandp.tile([128, 2048], BF16, tag="band", name="band1")
                    nc.sync.dma_start(
                        out=band, in_=y1r[bi, :, qt * 2048 : (qt + 1) * 2048]
                    )
                    vb1.append(band.rearrange("p (wo dw c) -> p wo dw c", wo=4, dw=8))
                    yield
                vb2 = []
                for hf in range(2):
                    band = bandp.tile([128, 2048], BF16, tag="band", name="band2")
                    nc.sync.dma_start(
                        out=band, in_=y2r[bi, :, hf * 2048 : (hf + 1) * 2048]
                    )
                    vb2.append(band.rearrange("p (wo dw c) -> p wo dw c", wo=4, dw=4))
                vb3 = []
                for hf in range(2):
                    band = bandp.tile([128, 2048], BF16, tag="band", name="band3")
                    nc.sync.dma_start(
                        out=band[:, 0:1280],
                        in_=y3r[bi, :, hf * 1280 : (hf + 1) * 1280],
                    )
                    vb3.append(
                        band[:, 0:1280].rearrange("p (wo dw c) -> p wo dw c", wo=2, dw=2)
                    )
                yield

                # x4 cols + xn4
                ln4c = colsp.tile([128, 2], F32, tag="ln4c", name="ln4c")
                nc.scalar.activation(out=ln4c, in_=mv4[:, :, 1], func=AF.Ln, bias=epscol)
                rst4 = colsp.tile([128, 2], F32, tag="rst4", name="rst4")
                nc.scalar.activation(out=rst4, in_=ln4c, func=AF.Exp, scale=-0.5)
                xn4 = work.tile([128, 2, C1], BF16, tag="xn4")
                for nt in range(2):
                    nc.vector.tensor_scalar(
                        xn4[:, nt], x_sb[:, nt],
                        mv4[:, nt, 0:1], rst4[:, nt : nt + 1],
                        ALU.subtract, ALU.mult,
                    )
                st["xn4"] = xn4
                yield

                # x transposes -> xT; q matmuls -> qT
                xT = work.tile([128, 4, NKV], BF16, tag="xT")
                for ck in range(4):
                    tp = pp.tile([128, 2, 128], BF16, tag="ppC", name="xtp", bufs=2)
                    for nt in range(2):
                        nc.tensor.transpose(
                            tp[:, nt], x_sb[:, nt, ck * 128 : (ck + 1) * 128], ident
                        )
                    nc.scalar.copy(out=xT[:, ck], in_=tp.rearrange("p a b -> p (a b)"))
                    yield
                qT = work.tile([128, 4, NKV], BF16, tag="qT", bufs=3)
                for mt in range(4):
                    qp = pp.tile([128, NKV], F32, tag="ppB", name="qp", bufs=2)
                    for kt in range(4):
                        nc.tensor.matmul(
                            qp,
                            wq_s[:, kt, mt * 128 : (mt + 1) * 128],
                            xT[:, kt],
                            start=(kt == 0),
                            stop=(kt == 3),
                        )
                    nc.scalar.copy(out=qT[:, mt], in_=qp)
                    yield
                st["qT"] = qT

                # ---- fused w+h-pool on the PE -> poolT (chan-major) ----
                # y1: accumulate the 4 remaining dw pairs per wo
                poolp1 = pp.tile([64, 16, 16], F32, tag="ppA", name="poolp1", bufs=2)
                for qt in range(4):
                    for wl in range(4):
                        for dw in range(8):
                            nc.tensor.matmul(
                                poolp1[:, qt * 4 + wl],
                                vb1[qt][:, wl, dw],
                                ah_s[0],
                                start=(dw == 0),
                                stop=(dw == 7),
                                skip_group_check=True,
                            )
                poolt1 = pooltp.tile([64, NKV], BF16, tag="poolt1")
                nc.scalar.copy(out=poolt1, in_=poolp1.rearrange("c a b -> c (a b)"))
                yield

                poolp2 = pp.tile([128, 2, 8, 16], F32, tag="ppA", name="poolp2", bufs=2)
                for hf in range(2):
                    for wl in range(4):
                        for dw in range(4):
                            nc.tensor.matmul(
                                poolp2[:, :, hf * 4 + wl],
                                vb2[hf][:, wl, dw],
                                ah_s[1],
                                start=(dw == 0),
                                stop=(dw == 3),
                                skip_group_check=True,
                            )
                poolt2 = pooltp.tile([128, NKV], BF16, tag="poolt2")
                nc.scalar.copy(out=poolt2, in_=poolp2.rearrange("c g a b -> c (g a b)"))
                yield

                poolt3 = pooltp.tile([128, 3, NKV], BF16, tag="poolt3")
                for cs in range(3):
                    cl = 64 if cs == 2 else 128
                    poolp3 = pp.tile([128, 4, 4, 16], F32, tag="ppA", name="poolp3", bufs=2)
                    for hf in range(2):
                        for wl in range(2):
                            for dw in range(2):
                                nc.tensor.matmul(
                                    poolp3[:cl, :, hf * 2 + wl],
                                    vb3[hf][:, wl, dw, cs * 128 : cs * 128 + cl],
                                    ah_s[2],
                                    start=(dw == 0),
                                    stop=(dw == 1),
                                    skip_group_check=True,
                                )
                    nc.scalar.copy(
                        out=poolt3[:cl, cs],
                        in_=poolp3[:cl].rearrange("c g a b -> c (g a b)"),
                    )
                    yield

                # ---- branch conv (token-major) + bn-stats LN + normalize
                poolts = [poolt1, poolt2, poolt3]
                xns = []
                if "conv" in ABLATE:
                    for br in range(3):
                        xn = xnp.tile([128, 2, C2[br]], BF16, tag=f"xn{br}", name=f"xn{br}")
                        nc.vector.memset(xn, 0.2)
                        xns.append(xn)
                    st["xns"] = xns
                    yield
                    return
                xns = [None, None, None]
                for br in [2, 1, 0]:
                    cb = C2[br]
                    nkt = (cb + 127) // 128
                    xn = xnp.tile([128, 2, cb], BF16, tag=f"xn{br}", name=f"xn{br}")
                    bst = colsp.tile([128, 2, 6], F32, tag=f"bst{br}", name=f"bst{br}")
                    mv = colsp.tile([128, 2, 2], F32, tag=f"mv{br}", name=f"mv{br}")
                    lnc = colsp.tile([128, 2], F32, tag=f"ln{br}", name=f"ln{br}")
                    rstd = colsp.tile([128, 2], F32, tag=f"rst{br}", name=f"rst{br}")
                    for tc in range(2):
                        prep = pp.tile([128, 320], F32, tag="ppA", name=f"prep{br}", bufs=2)
                        nc.tensor.matmul(
                            prep[:, 0:cb],
                            onesrow,
                            srb_s[br],
                            start=True,
                            stop=False,
                        )
                        for kt in range(nkt):
                            kl = min(128, cb - kt * 128)
                            if br < 2:
                                lhs = poolts[br][:kl, tc * 128 : (tc + 1) * 128]
                            else:
                                lhs = poolts[2][:kl, kt, tc * 128 : (tc + 1) * 128]
                            nc.tensor.matmul(
                                prep[:, 0:cb],
                                lhs,
                                srw_s[br][:kl, kt],
                                start=False,
                                stop=(kt == nkt - 1),
                            )
                        nc.vector.bn_stats(bst[:, tc], prep[:, 0:cb])
                        nc.vector.bn_aggr(mv[:, tc], bst[:, tc])
                        nc.scalar.activation(
                            out=lnc[:, tc : tc + 1], in_=mv[:, tc, 1:2],
                            func=AF.Ln, bias=epscol,
                        )
                        nc.scalar.activation(
                            out=rstd[:, tc : tc + 1], in_=lnc[:, tc : tc + 1],
                            func=AF.Exp, scale=-0.5,
                        )
                        nc.vector.tensor_scalar(
                            xn[:, tc], prep[:, 0:cb],
                            mv[:, tc, 0:1], rstd[:, tc : tc + 1],
                            ALU.subtract, ALU.mult,
                        )
                        yield
                    xns[br] = xn
                st["xns"] = xns

            def emit_s2a(bi, st):
                """Back-transposes + GELU -> xcT (contiguous Gelu-table block)."""
                xn4, xns = st["xn4"], st["xns"]
                xcT = work.tile([128, 8, NKV], BF16, tag="xcT")
                for ck in range(4):
                    tp4 = pp.tile([128, 2, 128], BF16, tag="ppC", name="tp4", bufs=2)
                    for nt in range(2):
                        nc.tensor.transpose(
                            tp4[:, nt], xn4[:, nt, ck * 128 : (ck + 1) * 128], ident
                        )
                    dst = xcT[:, 4 + ck]
                    nc.scalar.activation(
                        out=dst.rearrange("c (wo ho) -> c ho wo", wo=16),
                        in_=tp4.rearrange("c nt (hh wo) -> c (nt hh) wo", hh=8),
                        func=AF.Gelu,
                        scale=g_s[3][:, ck : ck + 1],
                        bias=b_s[3][:, ck : ck + 1],
                    )

                for br in [1, 2, 0]:
                    cb = C2[br]
                    xn = xns[br]
                    for ch in range(NPT[br]):
                        cl = min(128, cb - ch * 128)
                        kt_slot, base = XC_SLOT[br][ch]
                        tpb = pp.tile([128, 2, 128], BF16, tag="ppC", name=f"tpb{br}", bufs=2)
                        for tc in range(2):
                            nc.tensor.transpose(
                                tpb[:cl, tc],
                                xn[:, tc, ch * 128 : ch * 128 + cl],
                                ident,
                            )
                        nc.scalar.activation(
                            out=xcT[base : base + cl, kt_slot],
                            in_=tpb[:cl].rearrange("c a b -> c (a b)"),
                            func=AF.Gelu,
                            scale=g_s[br][0:cl, ch : ch + 1],
                            bias=b_s[br][0:cl, ch : ch + 1],
                        )
                st["xcT"] = xcT

            def s2b_gen(bi, st):
                """kv matmuls (chunked)."""
                xcT = st["xcT"]
                KTORD = [4, 5, 6, 7, 1, 2, 3, 0]
                kT = work.tile([128, 4, NKV], BF16, tag="kT")
                if "kv" in ABLATE:
                    nc.vector.memset(kT, 0.1)
                    st["kT"] = kT
                    v_aug = work.tile([128, 2, NH, HD + 1], BF16, tag="v_aug")
                    nc.vector.memset(v_aug, 0.1)
                    st["v_aug"] = v_aug
                    yield
                    return
                for mt in range(4):
                    kp = pp.tile([128, NKV], F32, tag="ppB", name="kp", bufs=2)
                    for i, kt in enumerate(KTORD):
                        nc.tensor.matmul(
                            kp,
                            wkv_s[:, kt, mt * 128 : (mt + 1) * 128],
                            xcT[:, kt],
                            start=(i == 0),
                            stop=(i == 7),
                        )
                    nc.vector.tensor_copy(kT[:, mt], kp)
                    yield
                st["kT"] = kT

                v_aug = work.tile([128, 2, NH, HD + 1], BF16, tag="v_aug")
                nc.vector.memset(v_aug[:, :, :, HD : HD + 1], 1.0)
                for mt in range(2):
                    for vh in range(2):
                        vp = pp.tile([128, NKV], F32, tag="ppB", name="vp", bufs=2)
                        for i, kt in enumerate(KTORD):
                            nc.tensor.matmul(
                                vp,
                                xcT[:, kt, mt * 128 : (mt + 1) * 128],
                                wkv_s[:, kt, 512 + vh * 256 : 768 + vh * 256],
                                start=(i == 0),
                                stop=(i == 7),
                            )
                        nc.vector.tensor_copy(
                            v_aug[:, mt, vh * 4 : (vh + 1) * 4, 0:HD],
                            vp.rearrange("p (h d) -> p h d", h=4),
                        )
                        yield
                st["v_aug"] = v_aug

            def s3_gen(bi, st):
                """Attention (head-pipelined) + proj + store."""
                qT, kT, v_aug = st["qT"], st["kT"], st["v_aug"]
                outT = work.tile([128, 4, NKV], BF16, tag="outT")
                if "attn" in ABLATE:
                    nc.vector.memset(outT, 0.5)
                    yield
                else:
                    sps, stes, pv2s, rss = {}, {}, {}, {}

                    def emit_sp(h):
                        j, hh = h // 2, h % 2
                        pb = hh * 64
                        sp = pp.tile([128, 2, NKV], F32, tag="ppC", name="sp", bufs=2)
                        for nt in range(2):
                            nc.tensor.matmul(
                                sp[:, nt],
                                kT[pb : pb + 64, j, nt * 128 : (nt + 1) * 128],
                                qT[pb : pb + 64, j],
                                start=True,
                                stop=True,
                                skip_group_check=True,
                            )
                        ste = step.tile([128, 2, NKV], BF16, tag="ste")
                        nc.scalar.activation(out=ste, in_=sp, func=AF.Exp, scale=SCALE)
                        stes[h] = ste

                    def emit_pv(h):
                        j, hh = h // 2, h % 2
                        if hh == 0:
                            pv2s[j] = pp.tile([65, 2, NKV], F32, tag="ppD", name="pv2", bufs=2)
                        for nt in range(2):
                            nc.tensor.matmul(
                                pv2s[j][:, hh],
                                v_aug[:, nt, h],
                                stes[h][:, nt],
                                start=(nt == 0),
                                stop=(nt == 1),
                                skip_group_check=True,
                            )
                        del stes[h]

                    def emit_norm(j):
                        pv2 = pv2s[j]
                        rs2 = rowsp.tile([1, 2, NKV], BF16, tag="rs2")
                        with nc.allow_low_precision(reason="bf16 softmax denom"):
                            nc.vector.reciprocal(rs2, pv2[64:65])
                        bc = pp.tile([128, NKV], F32, tag="ppB", name="bc", bufs=2)
                        nc.tensor.matmul(
                            bc[0:64], onesrow[:, 0:64], rs2[:, 0],
                            start=True, stop=True, skip_group_check=True,
                        )
                        nc.tensor.matmul(
                            bc[64:128], onesrow[:, 0:64], rs2[:, 1],
                            start=True, stop=True, skip_group_check=True,
                        )
                        bcs = step.tile([128, NKV], BF16, tag="bcs", name="bcs")
                        nc.scalar.copy(out=bcs, in_=bc)
                        for hh in range(2):
                            pb = hh * 64
                            nc.vector.scalar_tensor_tensor(
                                out=outT[pb : pb + 64, j],
                                in0=pv2[0:64, hh], scalar=1.0, in1=bcs[pb : pb + 64],
                                op0=ALU.mult, op1=ALU.mult,
                            )
                        del pv2s[j]

                    # head-level software pipeline: sp(h+1) issued between
                    # exp(h) and pv(h); pair tails interleave two heads later
                    emit_sp(0)
                    for h in range(NH):
                        if h + 1 < NH:
                            emit_sp(h + 1)
                        emit_pv(h)
                        if h >= 2 and h % 2 == 1:
                            emit_norm(h // 2 - 1)
                            yield
                    emit_norm(3)
                    yield

                osb = work.tile([128, 2, C1], BF16, tag="osb")
                for tc in range(2):
                    for fh in range(2):
                        fp = pp.tile([128, NKV], F32, tag="ppB", name="fp", bufs=2)
                        for kt in range(4):
                            nc.tensor.matmul(
                                fp,
                                outT[:, kt, tc * 128 : (tc + 1) * 128],
                                proj_s[:, kt, fh * 256 : (fh + 1) * 256],
                                start=(kt == 0),
                                stop=(kt == 3),
                            )
                        nc.vector.tensor_add(
                            osb[:, tc, fh * 256 : (fh + 1) * 256],
                            fp,
                            projb_s[:, fh * 256 : (fh + 1) * 256],
                        )
                    yield
                nc.scalar.dma_start(out=outr[bi], in_=osb)

            def _drain(g):
                if g is None:
                    return False
                try:
                    next(g)
                    return True
                except StopIteration:
                    return False

            # ---- software pipeline ------------------------------------
            # Window t: [gelu block of batch t-1 (contiguous, Gelu table)]
            # then round-robin chunks of S3(t-2) / kv(t-1) / S1(t)
            # (Exp/Ln-table + copies).  2 activation-table loads per window.
            NB = reps * BPC
            states = {}
            for t in range(NB + 2):
                if t < NB:
                    states[t] = {}
                if 1 <= t and t - 1 < NB:
                    emit_s2a((t - 1) % BPC, states[t - 1])
                gens = []
                if t >= 2:
                    gens.append(s3_gen((t - 2) % BPC, states[t - 2]))
                if 1 <= t and t - 1 < NB:
                    gens.append(s2b_gen((t - 1) % BPC, states[t - 1]))
                if t < NB:
                    gens.append(s1_gen(t % BPC, states[t]))
                while gens:
                    nxt = []
                    for g in gens:
                        try:
                            next(g)
                            nxt.append(g)
                        except StopIteration:
                            pass
                    gens = nxt
                if t >= 2:
                    del states[t - 2]

    _split_excess_waits(nc)
    return nc


def _prep_common(inputs):
    Wq = np.asarray(inputs["Wq"], dtype=np.float32)
    Wkv = np.asarray(inputs["Wkv"], dtype=np.float32)
    proj_w = np.asarray(inputs["proj_w"], dtype=np.float32)
    proj_b = np.asarray(inputs["proj_b"], dtype=np.float32)

    bf = ml_dtypes.bfloat16
    common = {
        "wq_t": np.ascontiguousarray(Wq.T).astype(bf),
        "wkv_t": np.ascontiguousarray(Wkv.T[_PERM, :]).astype(bf),
        "proj_t": np.ascontiguousarray(proj_w.T).astype(bf),
        "projb": proj_b.astype(bf),
    }
    ah = _pool_mats()
    for i in range(3):
        common[f"ah{i+1}"] = ah[i].astype(bf)
        c = C2[i]
        cpad = ((c + 127) // 128) * 128
        pr = min(c, 128)
        nkt = (c + 127) // 128
        srw_t = np.asarray(inputs[f"sr{i+1}_w"], dtype=np.float32).T  # [c_in, c_out]
        srw_p = np.zeros((nkt * pr, c), dtype=np.float32)
        srw_p[:c] = srw_t
        common[f"srw{i+1}_t"] = srw_p.astype(bf)
        common[f"srb{i+1}"] = np.asarray(
            inputs[f"sr{i+1}_b"], dtype=np.float32
        ).astype(bf)
    for i, c in enumerate((64, 128, 320, 512)):
        cpad = ((c + 127) // 128) * 128
        if i < 3:
            g = np.asarray(inputs[f"ln{i+1}_g"], dtype=np.float32)
            b = np.asarray(inputs[f"ln{i+1}_b"], dtype=np.float32)
        else:
            g = np.asarray(inputs["ln4_g"], dtype=np.float32)
            b = np.asarray(inputs["ln4_b"], dtype=np.float32)
        gp = np.zeros(cpad, dtype=np.float32)
        gp[:c] = g
        bp = np.zeros(cpad, dtype=np.float32)
        bp[:c] = b
        common[f"g{i+1}"] = gp
        common[f"lb{i+1}"] = bp
    return common


def kernel(**inputs):
    bf = ml_dtypes.bfloat16
    x = np.ascontiguousarray(inputs["x"]).astype(bf)
    y1 = np.ascontiguousarray(inputs["y1"]).astype(bf)
    y2 = np.ascontiguousarray(inputs["y2"]).astype(bf)
    y3 = np.ascontiguousarray(inputs["y3"]).astype(bf)
    common = _prep_common(inputs)

    nc = build_module()
    in_maps = []
    for c in range(NCORES):
        sl = slice(c * BPC, (c + 1) * BPC)
        m = dict(common)
        m["x"] = x[sl]
        m["y1"] = y1[sl]
        m["y2"] = y2[sl]
        m["y3"] = y3[sl]
        in_maps.append(m)

    res = run_bass_kernel_spmd(nc, in_maps, core_ids=list(range(NCORES)))
    return np.concatenate(
        [np.asarray(r["out"]).astype(np.float32) for r in res.results], axis=0
    )


if __name__ == "__main__":
    pass
